# revision 1
# baseline (speedup 1.0000x reference)
"""DAHead (dual attention head) Trainium2 Bass kernel.

Sharding: 8 cores = (batch b in 0..3) x (image half: rows 0-31 / 32-63).
Each core:
  - conv3x3(512->128) + BN + ReLU for both the PAM and CAM branches over its
    half's rows (+1 halo row each side), from a host-pre-padded x slice.
  - PAM: local q; k / d^T AllGathered across the pair; position attention
    computed in S^T = k^T q layout (keys on partitions) so no transposes are
    needed; exp on ACT, column sums on DVE, unnormalized e via PE, normalize
    via a rank-1 broadcast matmul.
  - CAM: local gram matrix AllReduced across the pair; softmax fully local.
  - conv3x3(128->32) + BN + ReLU on both branches, summed, written out.

Spatial layout on chip: width padded 64->66 with zero columns so 3x3 conv taps
are pure flat-offset reads; row halos materialized in the buffers.
"""

import os

import numpy as np

import concourse.bass as bass
import concourse.mybir as mybir
import concourse.tile as tile
from concourse.bass_utils import run_bass_kernel_spmd
from concourse.vector_clock import ScopedClock

FP32 = mybir.dt.float32
FP32R = mybir.dt.float32r
BF16 = mybir.dt.bfloat16
AF = mybir.ActivationFunctionType

NCORES = 8
B, CIN, H, W = 4, 512, 64, 64
C1 = 128          # conv1 out channels
C8 = 16           # q/k channels
CO = 32           # conv2 out channels
WP = W + 2        # padded width 66
XR = 36           # x rows in padded buffer (32 + 2 halo + 2 conv pad)
ER = 34           # "extended" output rows (32 + 1 halo each side)
NE = ER * WP      # 2244 flat ext positions
NX = XR * WP      # 2376 flat x positions
NLOC = 32 * W     # 2048 valid key positions per core
NFULL = 64 * W    # 4096 total key positions
EPS = 1e-5

# conv/PAM chunks over the flat ext space [1, 2243)
CH1 = [(1, 512), (513, 512), (1025, 512), (1537, 512), (2049, 194)]
# conv2 row-aligned chunks: (r0, nrows)
CH2 = [(0, 7), (7, 7), (14, 7), (21, 7), (28, 4)]

PAIRS = [[0, 1], [2, 3], [4, 5], [6, 7]]

_legalize_counter = [0]


def _patched_drain_and_barrier(self, tick_clock, wait_clock):
    """Tail drain split into single-wait drains (this walrus build encodes at
    most one sync wait per instruction)."""
    drain_inst = self.nc.sync.drain()
    wait_clock.add_sem_waits(
        drain_inst.ins, ScopedClock({None: tick_clock.global_clock})
    )
    si = drain_inst.ins.sync_info
    waits = list(si.on_wait) if si is not None else []
    if len(waits) > 1:
        si.on_wait = waits[:1]
        for i in range(1, len(waits)):
            extra = self.nc.sync.drain()
            extra.ins.sync_info = mybir.SyncInfo(on_wait=[waits[i]], on_update=[])
    self.nc.all_engine_barrier()
    assert self.sems is not None
    popped = self.nc._tile_sem_poison_stack.pop()
    assert popped is self._sem_poison
    self.nc.clear_and_free_semaphores(list(self.sems.allocated().values()))
    self.nc.all_engine_barrier()


tile.TileContext._drain_and_barrier = _patched_drain_and_barrier


def legalize_single_wait(nc):
    """Hoist extra sync waits onto same-engine EventSemaphore instructions so
    every instruction carries at most one wait."""
    n_split = 0
    for fn in nc.m.functions:
        for bb in fn.blocks:
            insts = bb.instructions
            out = []
            changed = False
            for inst in insts:
                si = getattr(inst, "sync_info", None)
                if si is not None and si.on_wait and len(si.on_wait) > 1:
                    waits = list(si.on_wait)
                    for w in waits[:-1]:
                        _legalize_counter[0] += 1
                        out.append(
                            mybir.InstEventSemaphore(
                                name=f"legwait-{_legalize_counter[0]}",
                                engine=inst.engine,
                                ins=[],
                                outs=[],
                                sync_info=mybir.SyncInfo(on_wait=[w], on_update=[]),
                            )
                        )
                        n_split += 1
                    si.on_wait = waits[-1:]
                    changed = True
                out.append(inst)
            if changed:
                insts[:] = out
    return n_split


def build_nc(variant="full", dump=False):
    nc = bass.Bass("TRN2", num_devices=NCORES)

    # -------- parameters (per-core views, host-packed) --------
    xs = nc.declare_dram_parameter("xs", [CIN, XR, WP], FP32, isOutput=False)
    w1pT = nc.declare_dram_parameter("w1pT", [CIN, 9 * C1], FP32, isOutput=False)
    w1cT = nc.declare_dram_parameter("w1cT", [CIN, 9 * C1], FP32, isOutput=False)
    w2pT = nc.declare_dram_parameter("w2pT", [C1 * 9, CO], FP32, isOutput=False)
    w2cT = nc.declare_dram_parameter("w2cT", [C1 * 9, CO], FP32, isOutput=False)
    wbT = nc.declare_dram_parameter("wbT", [C1, C8], FP32, isOutput=False)
    wcT = nc.declare_dram_parameter("wcT", [C1, C8], FP32, isOutput=False)
    wdT = nc.declare_dram_parameter("wdT", [C1, C1], FP32, isOutput=False)
    bb_p = nc.declare_dram_parameter("bb_p", [C8], FP32, isOutput=False)
    bc_p = nc.declare_dram_parameter("bc_p", [C8], FP32, isOutput=False)
    bd_p = nc.declare_dram_parameter("bd_p", [C1], FP32, isOutput=False)
    sc1p = nc.declare_dram_parameter("sc1p", [C1], FP32, isOutput=False)
    bi1p = nc.declare_dram_parameter("bi1p", [C1], FP32, isOutput=False)
    sc1c = nc.declare_dram_parameter("sc1c", [C1], FP32, isOutput=False)
    bi1c = nc.declare_dram_parameter("bi1c", [C1], FP32, isOutput=False)
    sc2p = nc.declare_dram_parameter("sc2p", [CO], FP32, isOutput=False)
    bi2p = nc.declare_dram_parameter("bi2p", [CO], FP32, isOutput=False)
    sc2c = nc.declare_dram_parameter("sc2c", [CO], FP32, isOutput=False)
    bi2c = nc.declare_dram_parameter("bi2c", [CO], FP32, isOutput=False)
    alpha_p = nc.declare_dram_parameter("alpha_p", [1], FP32, isOutput=False)
    beta_p = nc.declare_dram_parameter("beta_p", [1], FP32, isOutput=False)
    emask = nc.declare_dram_parameter("emask", [NE], FP32, isOutput=False)
    idm = nc.declare_dram_parameter("idm", [C1, C1], FP32, isOutput=False)
    onesc = nc.declare_dram_parameter("onesc", [C1], FP32, isOutput=False)
    outp = nc.declare_dram_parameter("out", [CO, 32, W], FP32, isOutput=True)
    if dump:
        d_yp = nc.declare_dram_parameter("d_yp", [C1, NE], FP32, isOutput=True)
        d_yc = nc.declare_dram_parameter("d_yc", [C1, NE], FP32, isOutput=True)
        d_k = nc.declare_dram_parameter("d_k", [C8, NFULL], FP32, isOutput=True)
        d_dT = nc.declare_dram_parameter("d_dT", [C1, 32 * C1], FP32, isOutput=True)
        d_g = nc.declare_dram_parameter("d_g", [C1, C1], FP32, isOutput=True)
        d_p = nc.declare_dram_parameter("d_p", [C1, NE], FP32, isOutput=True)
        d_c = nc.declare_dram_parameter("d_c", [C1, NE], FP32, isOutput=True)
        d_rm = nc.declare_dram_parameter("d_rm", [C1, 1], FP32, isOutput=True)
        d_gd = nc.declare_dram_parameter("d_gd", [C1, C1], FP32, isOutput=True)
        d_ge = nc.declare_dram_parameter("d_ge", [C1, C1], FP32, isOutput=True)
        d_at = nc.declare_dram_parameter("d_at", [C1, C1], FP32, isOutput=True)
        d_atT = nc.declare_dram_parameter("d_atT", [C1, C1], FP32, isOutput=True)

    with tile.TileContext(nc) as tc:
        px = tc.alloc_tile_pool(name="px", bufs=4)
        pw = tc.alloc_tile_pool(name="pw", bufs=1)
        pio = tc.alloc_tile_pool(name="pio", bufs=1)
        pe3 = tc.alloc_tile_pool(name="pe3", bufs=4)
        psm = tc.alloc_tile_pool(name="psm", bufs=2)
        dram = tc.alloc_tile_pool(name="dram", bufs=2, space="DRAM")
        # global PSUM plan (8 banks): sp-tag 2x[128,1024] (4) shared by conv
        # chunks / S-pairs / conv2; u 2; q 1 (q/z/k matmuls); m 1 (transposes,
        # gram group, attnT, cam chunks)
        ps_sp = tc.alloc_tile_pool(name="ps_sp", bufs=2, space="PSUM")
        ps_u = tc.alloc_tile_pool(name="ps_u", bufs=2, space="PSUM")
        ps_q = tc.alloc_tile_pool(name="ps_q", bufs=1, space="PSUM")
        ps_m = tc.alloc_tile_pool(name="ps_m", bufs=1, space="PSUM")

        # ---------------- loads ----------------
        w1pT_sb, w1cT_sb = [], []
        for c4 in range(4):
            t = pw.tile([C1, 9, C1], FP32R, tag=f"w1p{c4}")
            nc.sync.dma_start(
                out=t[:], in_=w1pT[c4 * C1:(c4 + 1) * C1, :].bitcast(FP32R)
            )
            w1pT_sb.append(t)

        x_sb = []
        x_r = []
        for c4 in range(4):
            xt = px.tile([C1, NX], FP32R, tag="xt")
            x_sb.append(xt)
            x_r.append(xt.rearrange("p (r c) -> p r c", c=WP))
        for g in range(3):
            for c4 in range(4):
                nc.sync.dma_start(
                    out=x_r[c4][:, 12 * g:12 * (g + 1), :],
                    in_=xs[c4 * C1:(c4 + 1) * C1, 12 * g:12 * (g + 1), :].bitcast(
                        FP32R
                    ),
                )

        for c4 in range(4):
            t = pw.tile([C1, 9, C1], FP32R, tag=f"w1c{c4}")
            nc.sync.dma_start(
                out=t[:], in_=w1cT[c4 * C1:(c4 + 1) * C1, :].bitcast(FP32R)
            )
            w1cT_sb.append(t)

        w2pT_sb = pw.tile([C1, 9, CO], FP32R, tag="w2p")
        nc.sync.dma_start(out=w2pT_sb[:], in_=w2pT[:].bitcast(FP32R))
        w2cT_sb = pw.tile([C1, 9, CO], FP32R, tag="w2c")
        nc.sync.dma_start(out=w2cT_sb[:], in_=w2cT[:].bitcast(FP32R))
        wbT_sb = pw.tile([C1, C8], FP32R, tag="wb")
        nc.sync.dma_start(out=wbT_sb[:], in_=wbT[:].bitcast(FP32R))
        wcT_sb = pw.tile([C1, C8], FP32R, tag="wc")
        nc.sync.dma_start(out=wcT_sb[:], in_=wcT[:].bitcast(FP32R))
        wdT_sb = pw.tile([C1, C1], FP32R, tag="wd")
        nc.sync.dma_start(out=wdT_sb[:], in_=wdT[:].bitcast(FP32R))
        id_sb = pw.tile([C1, C1], FP32, tag="id")
        nc.sync.dma_start(out=id_sb[:], in_=idm[:])

        def load_col(param, n, tag):
            t = pw.tile([n, 1], FP32, tag=tag)
            nc.sync.dma_start(out=t[:], in_=param[:].rearrange("(p o) -> p o", o=1))
            return t

        bb_sb = load_col(bb_p, C8, "bb")
        bc_sb = load_col(bc_p, C8, "bc")
        sc1p_sb = load_col(sc1p, C1, "sc1p")
        bi1p_sb = load_col(bi1p, C1, "bi1p")
        sc1c_sb = load_col(sc1c, C1, "sc1c")
        bi1c_sb = load_col(bi1c, C1, "bi1c")
        sc2p_sb = load_col(sc2p, CO, "sc2p")
        bi2p_sb = load_col(bi2p, CO, "bi2p")
        sc2c_sb = load_col(sc2c, CO, "sc2c")
        bi2c_sb = load_col(bi2c, CO, "bi2c")

        # broadcast loads (partition-replicated)
        alpha_sb = pw.tile([C1, 1], FP32, tag="alpha")
        nc.gpsimd.dma_start(
            out=alpha_sb[:],
            in_=bass.AP(tensor=alpha_p[:].tensor, offset=0, ap=[[0, C1], [1, 1]]),
        )
        beta_sb = pw.tile([C1, 1], FP32, tag="beta")
        nc.gpsimd.dma_start(
            out=beta_sb[:],
            in_=bass.AP(tensor=beta_p[:].tensor, offset=0, ap=[[0, C1], [1, 1]]),
        )
        bd_b = pw.tile([C1, C1], FP32, tag="bdb")
        nc.gpsimd.dma_start(
            out=bd_b[:],
            in_=bass.AP(tensor=bd_p[:].tensor, offset=0, ap=[[0, C1], [1, C1]]),
        )
        maskb = pw.tile([C1, NE], BF16, tag="mask")
        nc.gpsimd.dma_start(
            out=maskb[:],
            in_=bass.AP(tensor=emask[:].tensor, offset=0, ap=[[0, C1], [1, NE]]),
        )

        ones_col = pw.tile([C1, 1], BF16, tag="onec")
        nc.gpsimd.dma_start(
            out=ones_col[:], in_=onesc[:].rearrange("(p o) -> p o", o=1)
        )

        yp_pad = pio.tile([C1, NE], FP32R, tag="yp")
        yc_pad = pio.tile([C1, NE], FP32R, tag="yc")
        p_pad = px.tile([C1, NE], FP32R, tag="xt")
        c_pad = px.tile([C1, NE], FP32R, tag="xt")
        for t_ in (p_pad, c_pad):
            nc.vector.memset(t_[:, 0:1].bitcast(FP32), 0.0)
            nc.vector.memset(t_[:, NE - 1:NE].bitcast(FP32), 0.0)

        k_sb = pio.tile([C8, NFULL], FP32R, tag="ksb")
        dT_sb = pio.tile([C1, 32, C1], BF16, tag="dT")

        # DRAM bounce buffers for collectives
        mh_in0 = dram.tile([1280, C1], BF16, tag="mhi0")
        mh_in1 = dram.tile([1280, C1], BF16, tag="mhi1")
        mh_out0 = dram.tile([2560, C1], BF16, tag="mho0")
        mh_out1 = dram.tile([2560, C1], BF16, tag="mho1")
        mh_in = [mh_in0, mh_in1]
        mh_out = [mh_out0, mh_out1]
        gb_in = dram.tile([C1, C1], FP32, tag="gbi")
        gb_out = dram.tile([2 * C1, C1], FP32, tag="gbo")

        def conv1(w_sb, sc, bi, dst, chunks=None, dve_epilogue=False):
            for (s, n) in (chunks or CH1):
                cp = ps_sp.tile([C1, 2, 512], FP32, tag="sp")
                first = True
                for c4 in range(4):
                    for t in range(9):
                        ky, kx = t // 3, t % 3
                        off = s + ky * WP + kx - 1
                        nc.tensor.matmul(
                            cp[:, 0, :n],
                            w_sb[c4][:, t, :],
                            x_sb[c4][:, off:off + n],
                            start=first,
                            stop=(c4 == 3 and t == 8),
                        )
                        first = False
                if dve_epilogue:
                    nc.vector.tensor_scalar(
                        out=dst[:, s:s + n], in0=cp[:, 0, :n],
                        scalar1=sc[:], scalar2=bi[:],
                        op0=mybir.AluOpType.mult, op1=mybir.AluOpType.add,
                    )
                    nc.vector.tensor_scalar_max(
                        out=dst[:, s:s + n], in0=dst[:, s:s + n], scalar1=0.0
                    )
                else:
                    nc.scalar.activation(
                        dst[:, s:s + n], cp[:, 0, :n], AF.Relu,
                        bias=bi[:], scale=sc[:]
                    )

        if True:
            ypr = yp_pad.rearrange("p (r c) -> p r c", c=WP)
            ycr = yc_pad.rearrange("p (r c) -> p r c", c=WP)
            ypv = pio.tile([C1, NLOC], FP32R, tag="ypv")
            k_loc = pio.tile([C8, NLOC], FP32, tag="kloc")
            dT_loc = pio.tile([C1, 16, C1], BF16, tag="dTloc")

            def kd_half(hh):
                # valid rows 1+16*hh .. 17+16*hh of the ext buffer
                for j in (0, 1):
                    i = 2 * hh + j
                    nc.sync.dma_start(
                        out=ypv[:, 512 * i:512 * (i + 1)],
                        in_=ypr[:, 1 + 8 * i:9 + 8 * i, 1:65],
                    )
                for j in (0, 1):
                    i = 2 * hh + j
                    kp = ps_q.tile([C8, 512], FP32, tag="q")
                    nc.tensor.matmul(
                        kp[:],
                        wcT_sb[:],
                        ypv[:, 512 * i:512 * (i + 1)],
                        start=True,
                        stop=True,
                    )
                    nc.scalar.activation(
                        k_loc[:, 512 * i:512 * (i + 1)], kp[:],
                        AF.Identity, bias=bc_sb[:],
                    )
                for t in range(8 * hh, 8 * hh + 8):
                    dp = ps_m.tile([C1, C1], FP32, tag="m")
                    nc.tensor.matmul(
                        dp[:],
                        ypv[:, C1 * t:C1 * (t + 1)],
                        wdT_sb[:],
                        start=True,
                        stop=True,
                    )
                    nc.vector.tensor_add(out=dT_loc[:, t, :], in0=dp[:], in1=bd_b[:])
                nc.sync.dma_start(
                    out=bass.AP(
                        tensor=mh_in[hh][:].tensor,
                        offset=mh_in[hh][:].offset,
                        ap=[[C1, C1], [C1 * C1, 8], [1, C1]],
                    ),
                    in_=dT_loc[:, 8 * hh:8 * hh + 8, :],
                )
                nc.sync.dma_start(
                    out=bass.AP(
                        tensor=mh_in[hh][:].tensor,
                        offset=mh_in[hh][:].offset + 1024 * C1,
                        ap=[[1, 256 * C1]],
                    ),
                    in_=k_loc[:, 1024 * hh:1024 * (hh + 1)].bitcast(BF16),
                )
                if variant != "noccl":
                    nc.gpsimd.collective_compute(
                        "AllGather",
                        mybir.AluOpType.bypass,
                        replica_groups=PAIRS,
                        ins=[mh_in[hh][:].opt()],
                        outs=[mh_out[hh][:].opt()],
                    )
                else:
                    nc.sync.dma_start(out=mh_out[hh][:1280, :], in_=mh_in[hh][:])
                    nc.sync.dma_start(out=mh_out[hh][1280:, :], in_=mh_in[hh][:])

            # conv_p1 rows for k/dT half A (ext rows 1..17 need out flat
            # through 17*66+66=1188 -> chunks 0..2), then gather A while the
            # rest of conv_p1 runs.
            conv1(w1pT_sb, sc1p_sb, bi1p_sb, yp_pad, chunks=CH1[:3])
            kd_half(0)
            conv1(w1pT_sb, sc1p_sb, bi1p_sb, yp_pad, chunks=CH1[3:])
            kd_half(1)

            # ---- conv_c1 interleaved with CAM gram prep ----
            ycv = pio.tile([C1, NLOC], FP32R, tag="ycv")
            aT_sb = pio.tile([C1, 16, C1], FP32, tag="aTsb")

            def gram_prep(hh):
                for j in (0, 1):
                    i = 2 * hh + j
                    nc.sync.dma_start(
                        out=ycv[:, 512 * i:512 * (i + 1)],
                        in_=ycr[:, 1 + 8 * i:9 + 8 * i, 1:65],
                    )
                for t in range(8 * hh, 8 * hh + 8):
                    tp = ps_m.tile([C1, C1], FP32, tag="m")
                    nc.tensor.transpose(
                        tp[:], ycv[:, C1 * t:C1 * (t + 1)].bitcast(FP32), id_sb[:]
                    )
                    nc.vector.tensor_copy(out=aT_sb[:, t, :], in_=tp[:])

            conv1(w1cT_sb, sc1c_sb, bi1c_sb, yc_pad, chunks=CH1[:3],
                  dve_epilogue=True)
            gram_prep(0)
            conv1(w1cT_sb, sc1c_sb, bi1c_sb, yc_pad, chunks=CH1[3:],
                  dve_epilogue=True)
            gram_prep(1)

            gp = ps_m.tile([C1, C1], FP32, tag="m")
            for t in range(16):
                nc.tensor.matmul(
                    gp[:],
                    aT_sb[:, t, :],
                    aT_sb[:, t, :],
                    start=(t == 0),
                    stop=(t == 15),
                )
            g_loc = pio.tile([C1, C1], FP32, tag="gloc")
            nc.vector.tensor_copy(out=g_loc[:], in_=gp[:])
            nc.sync.dma_start(out=gb_in[:], in_=g_loc[:])
            if variant != "noccl":
                nc.gpsimd.collective_compute(
                    "AllGather",
                    mybir.AluOpType.bypass,
                    replica_groups=PAIRS,
                    ins=[gb_in[:].opt()],
                    outs=[gb_out[:].opt()],
                )
            else:
                nc.sync.dma_start(out=gb_out[:C1, :], in_=gb_in[:])
                nc.sync.dma_start(out=gb_out[C1:, :], in_=gb_in[:])

        # ---- gathered loads: half hh from each core h ----
        for hh in range(2):
            mho_f = mh_out[hh][:].bitcast(FP32R)
            for h in range(2):
                nc.sync.dma_start(
                    out=dT_sb[:, 16 * h + 8 * hh:16 * h + 8 * hh + 8, :],
                    in_=bass.AP(
                        tensor=mh_out[hh][:].tensor,
                        offset=mh_out[hh][:].offset + h * 1280 * C1,
                        ap=[[C1, C1], [C1 * C1, 8], [1, C1]],
                    ),
                )
                nc.sync.dma_start(
                    out=k_sb[:, NLOC * h + 1024 * hh:NLOC * h + 1024 * (hh + 1)],
                    in_=bass.AP(
                        tensor=mho_f.tensor,
                        offset=mho_f.offset + (h * 1280 + 1024) * (C1 // 2),
                        ap=[[1024, C8], [1, 1024]],
                    ),
                )
        g_full = pio.tile([C1, C1], FP32, tag="gfull")
        g_peer = pio.tile([C1, C1], FP32, tag="gpeer")
        nc.sync.dma_start(out=g_full[:], in_=gb_out[:C1, :])
        nc.sync.dma_start(out=g_peer[:], in_=gb_out[C1:, :])
        nc.vector.tensor_add(out=g_full[:], in0=g_full[:], in1=g_peer[:])

        if True:
            # ---------------- PAM ----------------
            hp = tc.high_priority()
            hp.__enter__()
            for (s, n) in CH1:
                qp = ps_q.tile([C8, 512], FP32, tag="q")
                nc.tensor.matmul(
                    qp[:, :n], wbT_sb[:], yp_pad[:, s:s + n], start=True, stop=True
                )
                q_sb = psm.tile([C8, 512], FP32R, tag="qsb")
                nc.vector.tensor_scalar_add(
                    out=q_sb[:, :n], in0=qp[:, :n], scalar1=bb_sb[0:C8]
                )
                nc.vector.tensor_mul(
                    out=q_sb[:, :n], in0=q_sb[:, :n], in1=maskb[0:C8, s:s + n]
                )

                zacc = psm.tile([C1, 2, 512], BF16, tag="zacc")
                up = ps_u.tile([C1, 512], FP32, tag="u")
                mp_first = True
                for mp in (0, 1, 2, 3, 8, 9, 10, 11, 4, 5, 6, 7, 12, 13, 14, 15):
                    sp = ps_sp.tile([C1, 2, 512], FP32, tag="sp")
                    for h in range(2):
                        nc.tensor.matmul(
                            sp[:, h, :n],
                            k_sb[:, (2 * mp + h) * C1:(2 * mp + h + 1) * C1],
                            q_sb[:, :n],
                            start=True,
                            stop=True,
                        )
                    et = pe3.tile([C1, 2, 512], BF16, tag="et")
                    nc.scalar.activation(et[:, :, :n], sp[:, :, :n], AF.Exp)
                    if mp_first:
                        nc.vector.tensor_copy(out=zacc[:, :, :n], in_=et[:, :, :n])
                    else:
                        nc.vector.tensor_add(
                            out=zacc[:, :, :n], in0=zacc[:, :, :n], in1=et[:, :, :n]
                        )
                    for h in range(2):
                        nc.tensor.matmul(
                            up[:, :n],
                            dT_sb[:, 2 * mp + h, :],
                            et[:, h, :n],
                            start=(mp_first and h == 0),
                            stop=(mp == 15 and h == 1),
                        )
                    mp_first = False
                nc.vector.tensor_add(
                    out=zacc[:, 0, :n], in0=zacc[:, 0, :n], in1=zacc[:, 1, :n]
                )
                zp = ps_q.tile([1, 512], FP32, tag="q")
                nc.tensor.matmul(
                    zp[:, :n], ones_col[:], zacc[:, 0, :n], start=True, stop=True
                )
                zr = psm.tile([1, 512], FP32, tag="zr")
                nc.vector.reciprocal(out=zr[:, :n], in_=zp[:, :n])
                nc.vector.tensor_scalar_mul(
                    out=zr[:, :n], in0=zr[:, :n], scalar1=alpha_sb[0:1]
                )
                zrd = dram.tile([1, 512], FP32, tag="zrd")
                nc.sync.dma_start(out=zrd[:, :n], in_=zr[:, :n])
                zbs = psm.tile([C1, 512], FP32, tag="zbs")
                nc.sync.dma_start(
                    out=zbs[:, :n],
                    in_=bass.AP(
                        tensor=zrd[:].tensor, offset=zrd[:].offset, ap=[[0, C1], [1, n]]
                    ),
                )
                t1 = psm.tile([C1, 512], FP32, tag="t1")
                nc.vector.tensor_mul(out=t1[:, :n], in0=up[:, :n], in1=zbs[:, :n])
                nc.vector.tensor_add(
                    out=t1[:, :n], in0=t1[:, :n], in1=yp_pad[:, s:s + n].bitcast(FP32)
                )
                nc.vector.tensor_mul(
                    out=p_pad[:, s:s + n], in0=t1[:, :n], in1=maskb[:, s:s + n]
                )

            hp.__exit__(None, None, None)

            # ---- conv_p2 (overlaps PAM tail; p chunks complete in order) ----
            out_p = pio.tile([CO, 34 * WP], FP32, tag="outp")
            for (r0, nr) in CH2:
                n2 = nr * WP - 2
                s2 = r0 * WP + 1
                o1p = ps_sp.tile([C1, 2, 512], FP32, tag="sp")
                for t in range(9):
                    ky, kx = t // 3, t % 3
                    off = s2 + ky * WP + kx - 1
                    nc.tensor.matmul(
                        o1p[:CO, 0, :n2],
                        w2pT_sb[:, t, :],
                        p_pad[:, off:off + n2],
                        start=(t == 0),
                        stop=(t == 8),
                    )
                nc.vector.tensor_scalar(
                    out=out_p[:, s2:s2 + n2], in0=o1p[:CO, 0, :n2],
                    scalar1=sc2p_sb[:], scalar2=bi2p_sb[:],
                    op0=mybir.AluOpType.mult, op1=mybir.AluOpType.add,
                )
                nc.vector.tensor_scalar_max(
                    out=out_p[:, s2:s2 + n2], in0=out_p[:, s2:s2 + n2], scalar1=0.0
                )

            # ---------------- CAM ----------------
            rowmax = pio.tile([C1, 1], FP32, tag="rmax")
            nc.vector.tensor_reduce(
                out=rowmax[:], in_=g_full[:], op=mybir.AluOpType.min,
                axis=mybir.AxisListType.X,
            )
            gdiff = pio.tile([C1, C1], FP32, tag="gdiff")
            nc.vector.tensor_scalar(
                out=gdiff[:], in0=g_full[:], scalar1=rowmax[:], scalar2=None,
                op0=mybir.AluOpType.subtract,
            )
            nc.vector.tensor_scalar_min(out=gdiff[:], in0=gdiff[:], scalar1=80.0)
            gexp = pio.tile([C1, C1], FP32, tag="gexp")
            nc.scalar.activation(gexp[:], gdiff[:], AF.Exp, scale=-1.0)
            rowsum = pio.tile([C1, 1], FP32, tag="rsum")
            nc.vector.reduce_sum(out=rowsum[:], in_=gexp[:], axis=mybir.AxisListType.X)
            rinv = pio.tile([C1, 1], FP32, tag="rinv")
            nc.vector.reciprocal(out=rinv[:], in_=rowsum[:])
            attn = pio.tile([C1, C1], FP32, tag="attn")
            nc.vector.tensor_scalar_mul(out=attn[:], in0=gexp[:], scalar1=rinv[:])
            nc.vector.tensor_scalar_mul(out=attn[:], in0=attn[:], scalar1=beta_sb[:])
            atp = ps_m.tile([C1, C1], FP32, tag="m")
            nc.tensor.transpose(atp[:], attn[:], id_sb[:])
            attnT = pio.tile([C1, C1], FP32R, tag="attnTs")
            nc.vector.tensor_copy(out=attnT[:], in_=atp[:])

            for (s, n) in CH1:
                cm = ps_m.tile([C1, 512], FP32, tag="m")
                nc.tensor.matmul(
                    cm[:, :n], attnT[:], yc_pad[:, s:s + n], start=True, stop=True
                )
                t2 = psm.tile([C1, 512], FP32, tag="t2")
                nc.vector.tensor_add(
                    out=t2[:, :n], in0=cm[:, :n], in1=yc_pad[:, s:s + n].bitcast(FP32)
                )
                nc.vector.tensor_mul(
                    out=c_pad[:, s:s + n], in0=t2[:, :n], in1=maskb[:, s:s + n]
                )

        if dump:
            nc.sync.dma_start(out=d_rm[:], in_=rowmax[:])
            nc.sync.dma_start(out=d_gd[:], in_=gdiff[:])
            nc.sync.dma_start(out=d_ge[:], in_=gexp[:])
            nc.sync.dma_start(out=d_at[:], in_=attn[:])
            nc.sync.dma_start(out=d_atT[:], in_=attnT[:].bitcast(FP32))
            nc.sync.dma_start(out=d_yp[:], in_=yp_pad[:].bitcast(FP32))
            nc.sync.dma_start(out=d_yc[:], in_=yc_pad[:].bitcast(FP32))
            nc.sync.dma_start(out=d_k[:], in_=k_sb[:].bitcast(FP32))
            nc.gpsimd.dma_start(out=d_dT[:], in_=dT_sb[:].opt())
            nc.sync.dma_start(out=d_g[:], in_=g_full[:])
            nc.sync.dma_start(out=d_p[:], in_=p_pad[:].bitcast(FP32))
            nc.sync.dma_start(out=d_c[:], in_=c_pad[:].bitcast(FP32))

        # ---------------- conv2 (c-branch + final add) ----------------
        if True:
            for ci2, (r0, nr) in enumerate(CH2):
                n2 = nr * WP - 2
                s2 = r0 * WP + 1
                o2p = ps_sp.tile([C1, 2, 512], FP32, tag="sp")
                for t in range(9):
                    ky, kx = t // 3, t % 3
                    off = s2 + ky * WP + kx - 1
                    nc.tensor.matmul(
                        o2p[:CO, 0, :n2],
                        w2cT_sb[:, t, :],
                        c_pad[:, off:off + n2],
                        start=(t == 0),
                        stop=(t == 8),
                    )
                o2 = psm.tile([CO, 512], FP32, tag="o2s")
                nc.vector.tensor_scalar(
                    out=o2[:, :n2], in0=o2p[:CO, 0, :n2],
                    scalar1=sc2c_sb[:], scalar2=bi2c_sb[:],
                    op0=mybir.AluOpType.mult, op1=mybir.AluOpType.add,
                )
                nc.vector.tensor_scalar_max(
                    out=o2[:, :n2], in0=o2[:, :n2], scalar1=0.0
                )
                nc.vector.tensor_add(
                    out=out_p[:, s2:s2 + n2], in0=out_p[:, s2:s2 + n2],
                    in1=o2[:, :n2],
                )
                opr = out_p.rearrange("p (r c) -> p r c", c=WP)
                nc.sync.dma_start(
                    out=outp[:, r0:r0 + nr, :], in_=opr[:, r0:r0 + nr, 1:65]
                )

        for p in (ps_m, ps_q, ps_u, ps_sp, dram, psm, pe3, pio, pw, px):
            p.release()

    legalize_single_wait(nc)

    # The neuron compile cache keys on the HLO, which does NOT include the
    # bass_exec backend_config (the BIR). Declare an unused input whose SHAPE
    # encodes a hash of the built module so any kernel change produces a new
    # cache key instead of silently reusing a stale NEFF.
    import hashlib
    h = int.from_bytes(
        hashlib.sha256(nc.to_json_bytes()).digest()[:4], "little"
    )
    nonce_len = 1 + (h % 4096)
    nc.declare_dram_parameter("nonce", [nonce_len], FP32, isOutput=False)
    nc._nonce_len = nonce_len
    return nc


def pack_inputs(inputs):
    """Host-side packing: per-core input maps."""
    x = np.asarray(inputs["x"], dtype=np.float32)

    def t1(w):  # [O,CI,3,3] -> [CI, 9*O] with layout [ci][ky*3+kx][o]
        w = np.asarray(w, dtype=np.float32)
        o = w.shape[0]
        return np.ascontiguousarray(
            w.transpose(1, 2, 3, 0).reshape(w.shape[1], 9 * o)
        )

    def bnsb(g, b, m, v):
        g, b, m, v = (np.asarray(a, dtype=np.float32) for a in (g, b, m, v))
        sc = g / np.sqrt(v + EPS)
        return sc, b - m * sc

    sc1p_, bi1p_ = bnsb(inputs["gp1"], inputs["bp1"], inputs["mp1"], inputs["vp1"])
    sc1c_, bi1c_ = bnsb(inputs["gc1"], inputs["bc1"], inputs["mc1"], inputs["vc1"])
    sc2p_, bi2p_ = bnsb(inputs["gp2"], inputs["bp2"], inputs["mp2"], inputs["vp2"])
    sc2c_, bi2c_ = bnsb(inputs["gc2"], inputs["bc2"], inputs["mc2"], inputs["vc2"])

    shared = {
        "w1pT": t1(inputs["wp1"]),
        "w1cT": t1(inputs["wc1"]),
        "w2pT": np.ascontiguousarray(
            np.asarray(inputs["wp2"], dtype=np.float32)
            .transpose(1, 2, 3, 0)
            .reshape(C1 * 9, CO)
        ),
        "w2cT": np.ascontiguousarray(
            np.asarray(inputs["wc2"], dtype=np.float32)
            .transpose(1, 2, 3, 0)
            .reshape(C1 * 9, CO)
        ),
        "wbT": np.ascontiguousarray(np.asarray(inputs["pam_wb"], np.float32).T),
        "wcT": np.ascontiguousarray(np.asarray(inputs["pam_wc"], np.float32).T),
        "wdT": np.ascontiguousarray(np.asarray(inputs["pam_wd"], np.float32).T),
        "bb_p": np.asarray(inputs["pam_bb"], np.float32),
        "bc_p": np.asarray(inputs["pam_bc"], np.float32),
        "bd_p": np.asarray(inputs["pam_bd"], np.float32),
        "sc1p": sc1p_, "bi1p": bi1p_, "sc1c": sc1c_, "bi1c": bi1c_,
        "sc2p": sc2p_, "bi2p": bi2p_, "sc2c": sc2c_, "bi2c": bi2c_,
        "alpha_p": np.asarray(inputs["alpha"], np.float32),
        "beta_p": np.asarray(inputs["beta"], np.float32),
        "idm": np.eye(C1, dtype=np.float32),
        "onesc": np.ones(C1, dtype=np.float32),
    }

    in_maps = []
    for core in range(NCORES):
        b, hf = core // 2, core % 2
        xsl = np.zeros((CIN, XR, WP), np.float32)
        if hf == 0:
            xsl[:, 2:36, 1:65] = x[b, :, 0:34, :]
        else:
            xsl[:, 0:34, 1:65] = x[b, :, 30:64, :]
        em = np.zeros((ER, WP), np.float32)
        if hf == 0:
            em[1:34, 1:65] = 1.0
        else:
            em[0:33, 1:65] = 1.0
        m = dict(shared)
        m["xs"] = xsl
        m["emask"] = em.reshape(-1)
        in_maps.append(m)
    return in_maps


def unpack_outputs(results):
    out = np.zeros((B, CO, H, W), np.float32)
    for core in range(NCORES):
        b, hf = core // 2, core % 2
        out[b, :, hf * 32:(hf + 1) * 32, :] = results[core]["out"]
    return out


_NC_CACHE = [None]


def kernel(**inputs) -> np.ndarray:
    # the axon NTFF trace hook module is absent here; make sure a stray
    # BASS_TRACE env var cannot route run_bass_kernel_spmd into it
    os.environ["BASS_NEVER_TRACE"] = "1"
    if _NC_CACHE[0] is None:
        _NC_CACHE[0] = build_nc()
    nc = _NC_CACHE[0]
    in_maps = pack_inputs(inputs)
    nz = np.zeros([getattr(nc, "_nonce_len", 1)], np.float32)
    for m in in_maps:
        m["nonce"] = nz
    res = run_bass_kernel_spmd(nc, in_maps, list(range(NCORES)), trace=False)
    return unpack_outputs(res.results)

